# revision 2
# baseline (speedup 1.0000x reference)
"""JPEG encoder Bass kernel v2 for TRN2 — 8-core data-parallel, DMA-roofline.

Math: per 8x8 block, dct = D@(X-128)@D.T = kron(D,D)[zz]@(x-128); fold the
whole thing into one K<=128 contraction with output features f=(c,zz) on
PSUM PARTITIONS (so per-partition Q scaling works), instances (n,b) on free.

Traffic compression (rel-err gate is 2e-2):
  - input img as uint16 fixed-point u=rint(img*257) (2B/px, quant err 25x
    smaller than fp16); the 1/257 is folded into the matmul constants.
  - no_quan output as fp16 (rel ~2e-4), quant output as int8 (max|q|=35,
    exact integers).

Per-core pipeline (64 batches), per rb (8 row-blocks of 8 image rows x 2
halves), per group g (4 of 16 column-blocks):
  u16 strip DMA -> 6 PE transposes (strided views; pixels onto partitions,
  u16 in, f32 PSUM out = exact integers) -> one DVE/ACT cast copy to f32r
  SBUF -> 3 f32r matmuls (f on partitions: K=128 c0|c1 N=512; c2 via two
  zero-padded K=128 N=256 into partition halves) -> one ACT bias copy
  (->fp16 nq) + one DVE scale/bias op (->int8 qq) -> staged contiguous
  output DMAs (host reassembles layout).
"""

import numpy as np
import concourse.mybir as mybir
import concourse.tile as tile
from concourse import bacc
from concourse.bass_utils import run_bass_kernel_spmd
from concourse.masks import make_identity

F32 = mybir.dt.float32
F32R = mybir.dt.float32r
F16 = mybir.dt.float16
U16 = mybir.dt.uint16
I8 = mybir.dt.int8

P = 8
B, C, H, W = 512, 3, 128, 128
NCORES = 8
BSH = B // NCORES          # 64 batches per core
NRB = 8                    # row-block groups (8 rows x 2 halves each)
NG = 4                     # cb-groups per rb (4 cb each)
MAGIC = 12582912.0         # 1.5 * 2**23

# qq rounding: 'cast' = rely on f32->int8 convert rounding to nearest;
# 'magic' = explicit (x+1.5*2^23)-1.5*2^23 on two ops.
QQ_MODE = "cast"

# buffer depths (tunable)
BUFS = {"imgp": 7, "rgp": 4, "xp": 6, "nqs": 4, "qqs": 4, "pt": 4, "po": 2}
ABLATE = set()  # dev-only: {"qq", "nq", "outdma", "mm", "pix"}
REG_ON_POOL = True
REG_AHEAD = True
PF_N = 5


def _zigzag_flat_idx(n=P):
    order = []
    for s in range(2 * n - 1):
        cells = [(r, s - r) for r in range(max(0, s - n + 1), min(s, n - 1) + 1)]
        if s % 2 == 0:
            cells.reverse()
        order.extend(cells)
    return np.array([r * n + c for r, c in order], dtype=np.int32)


def _build_consts(D: np.ndarray, Q: np.ndarray):
    ZZ = _zigzag_flat_idx()
    D64 = D.astype(np.float64)
    KD = np.kron(D64, D64)[ZZ, :]            # (64 zz, 64 pix)
    KDs = (KD / 256.0).T.astype(np.float16)  # (64 pix, 64 zz), u16 scale folded
    L01 = np.zeros((128, 128), dtype=np.float16)
    L01[0:64, 0:64] = KDs
    L01[64:128, 64:128] = KDs
    L2TB = np.zeros((128, 128), dtype=np.float16)
    L2TB[0:64, 0:64] = KDs       # cols 0-63: lhsT for parity-A (pix rows 0-63)
    L2TB[64:128, 64:128] = KDs   # cols 64-127: parity-B (pix rows 64-127)
    q_zz = Q.astype(np.float64).flatten()[ZZ]
    s = (1.0 / q_zz).astype(np.float32)      # (64,)
    scl = np.zeros((128, 5), dtype=np.float32)
    scl[:, 0] = np.tile(s, 2)                              # q scale per f
    scl[0, 1] = -1024.0 * s[0]                             # q bias (DC shift)
    scl[64, 1] = -1024.0 * s[0]
    scl[0, 2] = -1024.0                                    # nq bias (DC shift)
    scl[64, 2] = -1024.0
    scl[0, 3] = -1024.0 * s[0] + MAGIC                     # q bias + MAGIC
    scl[64, 3] = -1024.0 * s[0] + MAGIC
    scl[:, 3] += np.where(scl[:, 3] == 0.0, MAGIC, 0.0)
    return L01, L2TB, scl


def _build_nc():
    nc = bacc.Bacc("TRN2", target_bir_lowering=False, debug=False)

    img = nc.dram_tensor("img", [BSH, C, H, W], F16, kind="ExternalInput")
    l01 = nc.dram_tensor("l01", [128, 128], F16, kind="ExternalInput")
    l2tb = nc.dram_tensor("l2tb", [128, 128], F16, kind="ExternalInput")
    scl = nc.dram_tensor("scl", [128, 5], F32, kind="ExternalInput")
    idt = nc.dram_tensor("idt", [128, 128], F16, kind="ExternalInput")
    nqo = nc.dram_tensor("nqo", [NRB, 128, 3072], F16, kind="ExternalOutput")
    qqo = nc.dram_tensor("qqo", [NRB, 128, 3072], I8, kind="ExternalOutput")

    Ident = mybir.ActivationFunctionType.Identity
    Mult = mybir.AluOpType.mult
    AddOp = mybir.AluOpType.add
    SubOp = mybir.AluOpType.subtract

    # h = brp*64 + rb*8 + i ; partition = (brp b); free = (c i w)
    imgv = img[:].rearrange("b c (brp rb i) w -> rb brp b c (i w)",
                            brp=2, rb=NRB, i=P)

    with tile.TileContext(nc) as tc:
        with (
            tc.tile_pool(name="const", bufs=1) as constp,
            tc.tile_pool(name="imgp", bufs=BUFS["imgp"]) as imgp,
            tc.tile_pool(name="rgp", bufs=BUFS["rgp"]) as rgp,
            tc.tile_pool(name="xp", bufs=BUFS["xp"]) as xp,
            tc.tile_pool(name="nqs", bufs=BUFS["nqs"]) as nqsp,
            tc.tile_pool(name="qqs", bufs=BUFS["qqs"]) as qqsp,
            tc.tile_pool(name="pt", bufs=BUFS["pt"], space="PSUM") as ptp,
            tc.tile_pool(name="po", bufs=BUFS["po"], space="PSUM") as pop,
        ):
            cscl = constp.tile([128, 5], F32)
            l01r = constp.tile([128, 128], F16)
            l2r = constp.tile([128, 128], F16)
            identu = constp.tile([128, 128], F16)
            nc.gpsimd.dma_start(out=identu[:], in_=idt[:])
            nc.gpsimd.dma_start(out=l01r[:], in_=l01[:])
            nc.gpsimd.dma_start(out=l2r[:], in_=l2tb[:])
            nc.gpsimd.dma_start(out=cscl[:], in_=scl[:])

            s_ap = cscl[:, 0:1]
            bq_ap = cscl[:, 1:2]
            bnq_ap = cscl[:, 2:3]
            bqm_ap = cscl[:, 3:4]
            zero_ap = cscl[:, 4:5]

            PREFETCH = PF_N

            def load_img(rb, split=False):
                IMG = imgp.tile([128, 3 * 1024], F16, tag="img", name="img")
                for brp in range(2):
                    ov = IMG[brp * 64:(brp + 1) * 64, :].rearrange(
                        "p (c f) -> p c f", c=3)
                    if split:
                        nc.sync.dma_start(out=ov[:, 0:2], in_=imgv[rb, brp, :, 0:2])
                        nc.sync.dma_start(out=ov[:, 2:3], in_=imgv[rb, brp, :, 2:3])
                    else:
                        nc.sync.dma_start(out=ov, in_=imgv[rb, brp])
                return IMG

            imgs = {rb: load_img(rb, split=(rb == 0)) for rb in range(PREFETCH)}

            def do_regroup(rb):
                # regroup to cb-major: R[:, cb*128 + c*64 + i*8 + j] (c0,c1)
                # and R[:, 2048 + cbp*128 + par*64 + i*8 + j] (c2)
                IMG = imgs[rb]
                R = rgp.tile([128, 3072], F16, tag="rg", name="rg")
                in01 = IMG[:, 0:2048].rearrange(
                    "p (c i cb j) -> p c i cb j", c=2, i=8, cb=16)
                out01 = R[:, 0:2048].rearrange(
                    "p (cb c i j) -> p c i cb j", cb=16, c=2, i=8)
                reg_eng = nc.gpsimd if (REG_ON_POOL and rb >= 2) else nc.vector
                reg_eng.tensor_copy(out01, in01)
                in2 = IMG[:, 2048:3072].rearrange(
                    "p (i cbp par j) -> p i cbp par j", i=8, cbp=8, par=2)
                out2 = R[:, 2048:3072].rearrange(
                    "p (cbp par i j) -> p i cbp par j", cbp=8, par=2, i=8)
                reg_eng.tensor_copy(out2, in2)
                return R

            Rs = {0: do_regroup(0)}

            def emit_qq(prb, pNQS, pQQS, g):
                nqs = pNQS[:, g * 768:(g + 1) * 768]
                qqs = pQQS[:, g * 768:(g + 1) * 768]
                if g == 1:
                    nc.scalar.activation(qqs, nqs, Ident,
                                         bias=zero_ap, scale=s_ap)
                else:
                    nc.vector.tensor_scalar(qqs, nqs, s_ap, None, Mult)
                if g % 2 == 1 and "outdma" not in ABLATE:
                    h0, h1 = (g - 1) * 768, (g + 1) * 768
                    nc.sync.dma_start(out=qqo[prb, :, h0:h1],
                                      in_=pQQS[:, h0:h1])

            pending = None
            for rb in range(NRB):
                if not REG_AHEAD and rb not in Rs:
                    Rs[rb] = do_regroup(rb)
                R = Rs.pop(rb)
                imgs.pop(rb)
                if rb + PREFETCH < NRB:
                    imgs[rb + PREFETCH] = load_img(rb + PREFETCH)
                NQS = nqsp.tile([128, 3072], F16, tag="nqs", name="nqs")
                QQS = qqsp.tile([128, 3072], I8, tag="qqs", name="qqs")
                POs = []
                for g in range(NG):
                    PT = ptp.tile([128, 768], F16)
                    for ci in range(4):
                        cb = g * 4 + ci
                        nc.tensor.transpose(
                            PT[:, ci * 128:(ci + 1) * 128],
                            R[:, cb * 128:(cb + 1) * 128],
                            identu[:],
                        )
                    for pi in range(2):
                        cbp = g * 2 + pi
                        nc.tensor.transpose(
                            PT[:, 512 + pi * 128:512 + (pi + 1) * 128],
                            R[:, 2048 + cbp * 128:2048 + (cbp + 1) * 128],
                            identu[:],
                        )
                    X = xp.tile([128, 768], F16, tag="x", name="x")
                    nc.vector.tensor_copy(X[:], PT[:].bitcast(U16))
                    PO = pop.tile([128, 1024], F32)
                    nc.tensor.matmul(PO[:, 0:512], l01r[:], X[:, 0:512],
                                     start=True, stop=True)
                    nc.tensor.matmul(PO[:, 512:768], l2r[:],
                                     X[:, 512:768], start=True, stop=True)
                    nqs = NQS[:, g * 768:(g + 1) * 768]
                    if "nq" not in ABLATE:
                        nc.scalar.activation(nqs, PO[:, 0:768], Ident,
                                             bias=bnq_ap, scale=1.0)
                    if REG_AHEAD and g == 1 and rb + 1 < NRB:
                        Rs[rb + 1] = do_regroup(rb + 1)
                    if g % 2 == 1 and "nq" not in ABLATE and "outdma" not in ABLATE:
                        h0, h1 = (g - 1) * 768, (g + 1) * 768
                        nc.sync.dma_start(out=nqo[rb, :, h0:h1],
                                          in_=NQS[:, h0:h1])
                if "qq" not in ABLATE:
                    for g in range(NG):
                        emit_qq(rb, NQS, QQS, g)

    nc.compile()
    return nc


_NC_CACHE = None


def _get_nc():
    global _NC_CACHE
    if _NC_CACHE is None:
        _NC_CACHE = _build_nc()
    return _NC_CACHE


def _reassemble(nq_cores, qq_cores):
    """nq_cores/qq_cores: per-core [8, 128, 3072] arrays -> (flatten, no_quan)."""
    def core_out(o):
        # cols: (g 4, [A 512 | B 256]) ; A idx = (ci 4, brp 2, b 64), f = p
        o = o.reshape(NRB, 128, NG, 6, 128)
        A = o[:, :, :, 0:4, :].reshape(NRB, 128, NG, 4, 2, 64)
        # (rb, f, g, ci, brp, b) -> n=(brp*8+rb)*16+g*4+ci -> (brp,rb,g,ci,b,f)
        A = A.transpose(4, 0, 2, 3, 5, 1).reshape(256, 64, 128)
        Bp = o[:, :, :, 4:6, :].reshape(NRB, 2, 64, NG, 2, 2, 64)
        # (rb, parity, f64, g, pi, brp, b) -> n=(brp*8+rb)*16+g*4+pi*2+parity
        Bp = Bp.transpose(5, 0, 3, 4, 1, 6, 2).reshape(256, 64, 64)
        return A, Bp

    outs = []
    for cores in (qq_cores, nq_cores):
        As, Bs = zip(*(core_out(o) for o in cores))
        Af = np.concatenate(As, axis=1)   # (256, 512, 128)
        Bf = np.concatenate(Bs, axis=1)   # (256, 512, 64)
        full = np.concatenate(
            [Af.astype(np.float32), Bf.astype(np.float32)], axis=2)
        outs.append(full)
    return outs[0], outs[1]


def kernel(img, D, Q):
    img = np.asarray(img, dtype=np.float32)
    u16 = np.rint(img * np.float32(256.0)).astype(np.uint16)
    L01, L2TB, scl = _build_consts(np.asarray(D, np.float32),
                                   np.asarray(Q, np.float32))
    nc = _get_nc()
    in_maps = [
        {"img": np.ascontiguousarray(u16[k * BSH:(k + 1) * BSH]).view(np.float16),
         "l01": L01, "l2tb": L2TB, "scl": scl,
         "idt": np.eye(128, dtype=np.float16)}
        for k in range(NCORES)
    ]
    res = run_bass_kernel_spmd(nc, in_maps, core_ids=list(range(NCORES)))
    flatten, no_quan = _reassemble(
        [r["nqo"] for r in res.results], [r["qqo"] for r in res.results])
    return (flatten, no_quan)
